# revision 15
# baseline (speedup 1.0000x reference)
"""Contextual attention kernel for Trainium2 (8 NeuronCores, data-parallel over batch).

Math (per batch b):
    Q = feaQK @ q_w.T + q_b
    k3 = conv1d(feaQK.T, cn3_w, SAME) + b3 ; k5 = conv1d(..., cn5_w) + b5
    K = [feaQK, k3, k5] @ k_w.T + k_b
    V = feaV @ v_w.T + v_b
    S = (Q @ K.T) / sqrt(D); mask keys >= seqlen with -inf
    out = softmax(S) @ V + V

Kernel strategy:
  * The convs + concat + K-projection collapse into a single width-5 stencil:
        K[s] = sum_{d=-2..2} feaQK[s+d] @ Wk[d] + kb_eff
    composed on the host (15 matmul-units of work -> 9).
  * All activations on-chip in transposed layout ([feature, seq]); no
    on-device transposes anywhere.
  * Q/K projections, scores, and PV run in fp8(e4m3) with
    perf_mode=DoubleRow: each matmul contracts 256 (2x128 chunk pairs) at
    ~2x bf16 ALU rate. Power-of-2 scale factors (exact in fp8) keep the
    tiny weights out of the subnormal range and every fp8-written tensor
    under the TRN e4m3 max of 240 (overflow would produce Inf, not
    saturation):
        x*2^4, wq/wk*2^12, QT/KT*2^5, ET*2^4 (folded into the exp bias as
        +4*ln2; cancels exactly against den = sum ET in the softmax ratio).
    Narrow psum groups (<256 cols) use plain fp8 matmuls (FWL beats
    DoubleRow's 256-col weight load there).
  * V projection stays bf16 (out ~= V + small attention average, so V's
    precision dominates the final error); V8 = fp8 copy of V feeds the PV
    moving operand.
  * Keys beyond seqlength are dead: K/scores/PV cover only the first
    ceil(seqlen/128) key chunks per batch slot; batches paired
    longest-with-shortest across cores keep the compile-time per-slot
    chunk counts small. Sub-chunk masking goes through the exp bias.
  * 16 batches -> 2 per core, full weights on every core.
"""

import numpy as np
import ml_dtypes

import concourse.bass as bass
from concourse import bacc
import concourse.tile as tile
from concourse import mybir

B, S, C, D = 16, 1024, 1024, 1024
P = 128
NCI, NDI, NKI, NQI, NSI = C // P, D // P, S // P, S // P, S // P
NF = 512  # matmul free dim (one PSUM bank of fp32)
PAD = 2
SP8 = 1040  # padded seq cols for fp8 x (mult of 16 for DR interleave APs)
LB = 2  # local batches per core
NCORES = 8
MASK_NEG = -60000.0
SCALE = 1.0 / 32.0  # 1/sqrt(D)

# fp8 power-of-2 scales (exact): see module docstring.
SX, SW, SQ, SK = 2.0**4, 2.0**12, 2.0**5, 2.0**5
EEXP = 4  # ET = 2^4 * exp(scores/32), via +EEXP*ln2 in the exp bias
QSCALE = SQ / (SX * SW)            # psum -> QT units
KSCALE = SK / (SX * SW)
ESCALE = SCALE / (SQ * SK)         # psum -> exp input
F8MAX = 240.0                      # TRN e4m3 max normal

BF = mybir.dt.bfloat16
F8 = mybir.dt.float8e4
F32 = mybir.dt.float32
AF = mybir.ActivationFunctionType
DR = mybir.MatmulPerfMode.DoubleRow
E4 = ml_dtypes.float8_e4m3

TRACE = False  # set by test harness to collect HW profile
_CACHE = {}


def _build_program(vs):
    nc = bacc.Bacc("TRN2", dynamic_dma_scratch_size=256)

    x8 = nc.dram_tensor("x8", [LB, C, SP8], F8, kind="ExternalInput")
    fvt = nc.dram_tensor("fvt", [LB, C, S], BF, kind="ExternalInput")
    wq8 = nc.dram_tensor("wq8", [C, D], F8, kind="ExternalInput")
    wk8 = nc.dram_tensor("wk8", [5, C, D], F8, kind="ExternalInput")
    wv = nc.dram_tensor("wv", [C, D], BF, kind="ExternalInput")
    qb = nc.dram_tensor("qb", [P, NDI], F32, kind="ExternalInput")
    kb = nc.dram_tensor("kb", [P, NDI], F32, kind="ExternalInput")
    vb = nc.dram_tensor("vb", [P, D], F32, kind="ExternalInput")
    mb = nc.dram_tensor("mb", [LB, P, NKI], F32, kind="ExternalInput")
    out = nc.dram_tensor("out", [LB, S, D], BF, kind="ExternalOutput")

    with tile.TileContext(nc) as tc:
        _emit(nc, tc, x8, fvt, wq8, wk8, wv, qb, kb, vb, mb, out, vs)
    nc.finalize()
    return nc


def _emit(nc, tc, x8, fvt, wq8, wk8, wv, qb, kb, vb, mb, out, vs):
    from contextlib import ExitStack

    with ExitStack() as ctx:
        wpool = ctx.enter_context(tc.tile_pool(name="wpool", bufs=1))
        apool = ctx.enter_context(tc.tile_pool(name="apool", bufs=1))
        opool = ctx.enter_context(tc.tile_pool(name="opool", bufs=3))
        spool = ctx.enter_context(tc.tile_pool(name="spool", bufs=2))
        pp = ctx.enter_context(tc.tile_pool(name="pp", bufs=6, space="PSUM"))
        pd = ctx.enter_context(tc.tile_pool(name="pd", bufs=2, space="PSUM"))

        QB = wpool.tile([P, NDI], F32, tag="qb")
        nc.sync.dma_start(out=QB, in_=qb[:, :])
        KB = wpool.tile([P, NDI], F32, tag="kb")
        nc.sync.dma_start(out=KB, in_=kb[:, :])
        VB = wpool.tile([P, D], F32, tag="vb")
        nc.sync.dma_start(out=VB, in_=vb[:, :])
        ONES = wpool.tile([P, 1], F8, tag="ones")
        nc.vector.memset(ONES, 1.0)
        WQ8 = wpool.tile([P, NCI, D], F8, tag="wq8")
        WV = wpool.tile([P, NCI, D], BF, tag="wv")
        WK8 = None

        for b in range(LB):
            v = vs[b]  # valid key chunks for this batch slot
            # key-dim psum groups: equal-width pieces covering v*128 cols
            # (equal widths keep every group wide enough for DoubleRow)
            n_g = -(-v * P // NF)
            base = (v * P // n_g) // 32 * 32
            kg, off = [], 0
            for g in range(n_g):
                w = v * P - off if g == n_g - 1 else base
                kg.append((off, w))
                off += w

            # --- stage Q: QT8[d, s] (fp8 DoubleRow over ci pairs) --------
            X8 = apool.tile([P, NCI, SP8], F8, tag="x8")
            nc.sync.dma_start(
                out=X8, in_=x8[b].rearrange("(ci p) s -> p ci s", p=P))
            if b == 0:
                # low di columns first so early psum groups can start
                # before the whole 1 MiB of wq8 lands
                for dq in range(2):
                    nc.sync.dma_start(
                        out=WQ8[:, :, dq * NF:(dq + 1) * NF],
                        in_=wq8[:, dq * NF:(dq + 1) * NF].rearrange(
                            "(ci p) d -> p ci d", p=P))
            MB = spool.tile([P, NKI], F32, tag="mb")
            nc.sync.dma_start(out=MB, in_=mb[b])
            QT8 = apool.tile([P, NDI, S], F8, tag="qt8")
            for di in range(NDI):
                ps = [pp.tile([P, NF], F32, tag="ps", name=f"ps{_i}") for _i in range(2)]
                for c0 in range(0, NCI, 2):
                    lhsT = WQ8[:, c0:c0 + 2, di * P:(di + 1) * P]
                    for sh in range(2):
                        nc.tensor.matmul(
                            ps[sh], lhsT,
                            X8[:, c0:c0 + 2, PAD + sh * NF: PAD + sh * NF + NF],
                            start=(c0 == 0), stop=(c0 == NCI - 2), perf_mode=DR)
                for sh in range(2):
                    nc.scalar.activation(
                        QT8[:, di, sh * NF:(sh + 1) * NF], ps[sh], AF.Identity,
                        bias=QB[:, di:di + 1], scale=QSCALE)

            # --- stage K: KT8[d, s] (width-5 stencil, v key chunks) ------
            if WK8 is None:
                WK8 = []
                for j in range(5):
                    t = wpool.tile([P, NCI, D], F8, tag=f"wk8{j}")
                    # second HWDGE ring (Activation) in parallel with sync's
                    nc.scalar.dma_start(
                        out=t, in_=wk8[j].rearrange("(ci p) d -> p ci d", p=P))
                    WK8.append(t)
            KT8 = apool.tile([P, NDI, S], F8, tag="kt8")
            for di in range(NDI):
                ps = [pp.tile([P, NF], F32, tag="ps", name=f"ps{_i}")
                      for _i in range(len(kg))]
                # per-group matmul counters for start/stop bookkeeping
                ndr = [w >= 160 for (_, w) in kg]
                total = [(5 * NCI // 2) if d else 5 * NCI for d in ndr]
                done = [0] * len(kg)
                for j in range(5):
                    for c0 in range(0, NCI, 2):
                        for g, (off, w) in enumerate(kg):
                            if ndr[g]:
                                nc.tensor.matmul(
                                    ps[g][:, :w],
                                    WK8[j][:, c0:c0 + 2, di * P:(di + 1) * P],
                                    X8[:, c0:c0 + 2, j + off: j + off + w],
                                    start=(done[g] == 0),
                                    stop=(done[g] == total[g] - 1),
                                    perf_mode=DR)
                                done[g] += 1
                            else:
                                for cc in (c0, c0 + 1):
                                    nc.tensor.matmul(
                                        ps[g][:, :w],
                                        WK8[j][:, cc, di * P:(di + 1) * P],
                                        X8[:, cc, j + off: j + off + w],
                                        start=(done[g] == 0),
                                        stop=(done[g] == total[g] - 1))
                                    done[g] += 1
                for g, (off, w) in enumerate(kg):
                    nc.scalar.activation(
                        KT8[:, di, off:off + w], ps[g][:, :w], AF.Identity,
                        bias=KB[:, di:di + 1], scale=KSCALE)

            # --- stage E: ET8[k, q] = 2^4 exp(scoresT/32 + mask) ---------
            ET8 = apool.tile([P, NKI, S], F8, tag="et8")
            for ki in range(v):
                ps = [pp.tile([P, NF], F32, tag="ps", name=f"ps{_i}") for _i in range(2)]
                for d0 in range(0, NDI, 2):
                    lhsT = KT8[:, d0:d0 + 2, ki * P:(ki + 1) * P]
                    for qh in range(2):
                        nc.tensor.matmul(
                            ps[qh], lhsT, QT8[:, d0:d0 + 2, qh * NF:(qh + 1) * NF],
                            start=(d0 == 0), stop=(d0 == NDI - 2), perf_mode=DR)
                for qh in range(2):
                    nc.scalar.activation(
                        ET8[:, ki, qh * NF:(qh + 1) * NF], ps[qh], AF.Exp,
                        bias=MB[:, ki:ki + 1], scale=ESCALE)

            # --- stage V: V natural [s, d] (bf16) + fp8 copy for PV ------
            FVT = apool.tile([P, NCI, S], BF, tag="fvt")
            nc.sync.dma_start(
                out=FVT, in_=fvt[b].rearrange("(ci p) s -> p ci s", p=P))
            if b == 0:
                nc.sync.dma_start(
                    out=WV, in_=wv.rearrange("(ci p) d -> p ci d", p=P))
            V = apool.tile([P, NSI, D], BF, tag="v")
            V8 = apool.tile([P, NKI, D], F8, tag="v8")
            for si in range(NSI):
                ps = [pp.tile([P, NF], F32, tag="ps", name=f"ps{_i}") for _i in range(2)]
                for ci in range(NCI):
                    lhsT = FVT[:, ci, si * P:(si + 1) * P]
                    for dh in range(2):
                        nc.tensor.matmul(
                            ps[dh], lhsT, WV[:, ci, dh * NF:(dh + 1) * NF],
                            start=(ci == 0), stop=(ci == NCI - 1))
                for dh in range(2):
                    nc.vector.tensor_add(
                        V[:, si, dh * NF:(dh + 1) * NF], ps[dh],
                        VB[:, dh * NF:(dh + 1) * NF])
                if si < v:
                    for dh in range(2):
                        nc.scalar.activation(
                            V8[:, si, dh * NF:(dh + 1) * NF],
                            V[:, si, dh * NF:(dh + 1) * NF],
                            AF.Copy, bias=0.0, scale=1.0)

            # --- stage F: out = (ET^T @ V) / den + V ---------------------
            for qi in range(NQI):
                pso = [pp.tile([P, NF], F32, tag="ps", name=f"pso{_i}") for _i in range(2)]
                psd = pd.tile([P, 1], F32, tag="den")
                # den first: its tiny psum evacuates (reciprocal) while the
                # pso DoubleRow matmuls still run, so the pd bank recycles
                # without ever stalling the PE.
                for ki in range(v):
                    nc.tensor.matmul(
                        psd, ET8[:, ki, qi * P:(qi + 1) * P], ONES,
                        start=(ki == 0), stop=(ki == v - 1))
                REC = spool.tile([P, 1], F32, tag="rec")
                nc.vector.reciprocal(REC, psd)
                vev = v - (v % 2)
                for k0 in range(0, vev, 2):
                    lhsT = ET8[:, k0:k0 + 2, qi * P:(qi + 1) * P]
                    st, sp_ = (k0 == 0), (k0 + 2 >= v)
                    for dh in range(2):
                        nc.tensor.matmul(
                            pso[dh], lhsT, V8[:, k0:k0 + 2, dh * NF:(dh + 1) * NF],
                            start=st, stop=sp_, perf_mode=DR)
                if v % 2:
                    lhsT = ET8[:, v - 1, qi * P:(qi + 1) * P]
                    for dh in range(2):
                        nc.tensor.matmul(
                            pso[dh], lhsT, V8[:, v - 1, dh * NF:(dh + 1) * NF],
                            start=(v == 1), stop=True)
                # Free the PSUM banks with plain DVE copies that wait only on
                # the matmul stop; the reciprocal-scale and +V run in place on
                # SBUF afterwards, off the PE-critical path.
                OTs = []
                for dh in range(2):
                    OT = opool.tile([P, NF], F32, tag="out", name=f"ot{dh}")
                    nc.vector.tensor_copy(OT, pso[dh])
                    OTs.append(OT)
                for dh in range(2):
                    OT = OTs[dh]
                    nc.scalar.activation(
                        OT, OT, AF.Copy, bias=0.0, scale=REC)
                    OB = opool.tile([P, NF], BF, tag="outb", name=f"ob{dh}")
                    nc.vector.tensor_add(
                        OB, OT, V[:, qi, dh * NF:(dh + 1) * NF])
                    nc.scalar.dma_start(
                        out=out[b, qi * P:(qi + 1) * P, dh * NF:(dh + 1) * NF],
                        in_=OB)


def _prep_host(feaQK, feaV, seqlengths, cn3_w, cn3_b, cn5_w, cn5_b,
               k_w, k_b, q_w, q_b, v_w, v_b):
    """Compose weights, assign batches to cores, lay out per-core inputs."""
    f32 = np.float32
    bf16 = ml_dtypes.bfloat16
    feaQK = np.asarray(feaQK, f32)
    feaV = np.asarray(feaV, f32)
    seqlengths = np.asarray(seqlengths).astype(np.int64)

    W1 = np.asarray(k_w, f32)[:, :C]
    W2 = np.asarray(k_w, f32)[:, C:2 * C]
    W3 = np.asarray(k_w, f32)[:, 2 * C:]

    wk = np.zeros((5, C, D), f32)  # [tap j (= shift+2), c, d]
    for t in range(3):
        wk[t + 1] += (W2 @ np.asarray(cn3_w, f32)[:, :, t]).T
    for t in range(5):
        wk[t] += (W3 @ np.asarray(cn5_w, f32)[:, :, t]).T
    wk[2] += W1.T
    kb_eff = (np.asarray(k_b, f32) + W2 @ np.asarray(cn3_b, f32)
              + W3 @ np.asarray(cn5_b, f32))

    def q8(a, s):
        return np.clip(np.asarray(a, f32) * s, -F8MAX, F8MAX).astype(E4)

    wq8 = np.ascontiguousarray(q8(np.asarray(q_w, f32).T, SW))
    wk8 = np.ascontiguousarray(q8(wk, SW))
    wv_b = np.ascontiguousarray(np.asarray(v_w, f32).T).astype(bf16)

    qb_pd = np.ascontiguousarray(
        (np.asarray(q_b, f32) * SQ).reshape(NDI, P).T)
    kb_pd = np.ascontiguousarray((kb_eff * SK).reshape(NDI, P).T)
    vb_rep = np.ascontiguousarray(
        np.broadcast_to(np.asarray(v_b, f32), (P, D)))

    key_valid = np.arange(S)[None, :] < seqlengths[:, None]
    mask = np.where(key_valid, EEXP * np.log(2.0), MASK_NEG).astype(f32)

    # Pair longest with shortest so the compile-time per-slot chunk counts
    # (max over cores) stay near the per-core optimum.
    vchunks = np.clip(np.ceil(seqlengths / P).astype(int), 1, NKI)
    order = np.argsort(-seqlengths, kind="stable")
    batch_of = np.zeros((NCORES, LB), int)
    for i in range(NCORES):
        batch_of[i, 0] = order[B - 1 - i]
        batch_of[i, 1] = order[i]
    vs = (int(vchunks[batch_of[:, 0]].max()),
          int(vchunks[batch_of[:, 1]].max()))

    in_maps = []
    for core in range(NCORES):
        bs = batch_of[core]
        x8s = np.zeros((LB, C, SP8), E4)
        x8s[:, :, PAD:PAD + S] = q8(feaQK[bs].transpose(0, 2, 1), SX)
        fvts = np.ascontiguousarray(
            feaV[bs].transpose(0, 2, 1)).astype(bf16)
        mbs = np.ascontiguousarray(
            mask[bs].reshape(LB, NKI, P).transpose(0, 2, 1))
        in_maps.append({
            "x8": x8s, "fvt": fvts,
            "wq8": wq8, "wk8": wk8, "wv": wv_b,
            "qb": qb_pd, "kb": kb_pd, "vb": vb_rep, "mb": mbs,
        })
    return in_maps, batch_of, vs


def kernel(**inputs):
    from concourse.bass_utils import run_bass_kernel_spmd

    in_maps, batch_of, vs = _prep_host(**inputs)
    if _CACHE.get("vs") != vs:
        _CACHE["nc"] = _build_program(vs)
        _CACHE["vs"] = vs
    nc = _CACHE["nc"]
    res = run_bass_kernel_spmd(nc, in_maps, core_ids=list(range(NCORES)),
                               trace=TRACE)
    _CACHE["last_result"] = res
    full = np.zeros((B, S, D), np.float32)
    for core in range(NCORES):
        full[batch_of[core]] = res.results[core]["out"].astype(np.float32)
    return full


# revision 17
# speedup vs baseline: 1.0188x; 1.0188x over previous
"""Contextual attention kernel for Trainium2 (8 NeuronCores, data-parallel over batch).

Math (per batch b):
    Q = feaQK @ q_w.T + q_b
    k3 = conv1d(feaQK.T, cn3_w, SAME) + b3 ; k5 = conv1d(..., cn5_w) + b5
    K = [feaQK, k3, k5] @ k_w.T + k_b
    V = feaV @ v_w.T + v_b
    S = (Q @ K.T) / sqrt(D); mask keys >= seqlen with -inf
    out = softmax(S) @ V + V

Kernel strategy:
  * The convs + concat + K-projection collapse into a single width-5 stencil:
        K[s] = sum_{d=-2..2} feaQK[s+d] @ Wk[d] + kb_eff
    composed on the host (15 matmul-units of work -> 9).
  * All activations on-chip in transposed layout ([feature, seq]); no
    on-device transposes anywhere.
  * Q/K projections, scores, and PV run in fp8(e4m3) with
    perf_mode=DoubleRow: each matmul contracts 256 (2x128 chunk pairs) at
    ~2x bf16 ALU rate. Power-of-2 scale factors (exact in fp8) keep the
    tiny weights out of the subnormal range and every fp8-written tensor
    under the TRN e4m3 max of 240 (overflow would produce Inf, not
    saturation):
        x*2^4, wq/wk*2^12, QT/KT*2^5, ET*2^4 (folded into the exp bias as
        +4*ln2; cancels exactly against den = sum ET in the softmax ratio).
    Narrow psum groups (<256 cols) use plain fp8 matmuls (FWL beats
    DoubleRow's 256-col weight load there).
  * V projection stays bf16 (out ~= V + small attention average, so V's
    precision dominates the final error); V8 = fp8 copy of V feeds the PV
    moving operand.
  * Keys beyond seqlength are dead: K/scores/PV cover only the first
    ceil(seqlen/128) key chunks per batch slot; batches paired
    longest-with-shortest across cores keep the compile-time per-slot
    chunk counts small. Sub-chunk masking goes through the exp bias.
  * 16 batches -> 2 per core, full weights on every core.
"""

import numpy as np
import ml_dtypes

import concourse.bass as bass
from concourse import bacc
import concourse.tile as tile
from concourse import mybir

B, S, C, D = 16, 1024, 1024, 1024
P = 128
NCI, NDI, NKI, NQI, NSI = C // P, D // P, S // P, S // P, S // P
NF = 512  # matmul free dim (one PSUM bank of fp32)
PAD = 2
SP8 = 1040  # padded seq cols for fp8 x (mult of 16 for DR interleave APs)
LB = 2  # local batches per core
NCORES = 8
MASK_NEG = -60000.0
SCALE = 1.0 / 32.0  # 1/sqrt(D)

# fp8 power-of-2 scales (exact): see module docstring.
SX, SW, SQ, SK = 2.0**4, 2.0**12, 2.0**5, 2.0**5
EEXP = 4  # ET = 2^4 * exp(scores/32), via +EEXP*ln2 in the exp bias
QSCALE = SQ / (SX * SW)            # psum -> QT units
KSCALE = SK / (SX * SW)
ESCALE = SCALE / (SQ * SK)         # psum -> exp input
F8MAX = 240.0                      # TRN e4m3 max normal

BF = mybir.dt.bfloat16
F8 = mybir.dt.float8e4
F32 = mybir.dt.float32
AF = mybir.ActivationFunctionType
DR = mybir.MatmulPerfMode.DoubleRow
E4 = ml_dtypes.float8_e4m3

TRACE = False  # set by test harness to collect HW profile
_CACHE = {}


def _build_program(vs):
    nc = bacc.Bacc("TRN2", dynamic_dma_scratch_size=256)

    x8 = nc.dram_tensor("x8", [LB, C, SP8], F8, kind="ExternalInput")
    fvt = nc.dram_tensor("fvt", [LB, C, S], BF, kind="ExternalInput")
    wq8 = nc.dram_tensor("wq8", [C, D], F8, kind="ExternalInput")
    wk8 = nc.dram_tensor("wk8", [5, C, D], F8, kind="ExternalInput")
    wv = nc.dram_tensor("wv", [C, D], BF, kind="ExternalInput")
    qb = nc.dram_tensor("qb", [P, NDI], F32, kind="ExternalInput")
    kb = nc.dram_tensor("kb", [P, NDI], F32, kind="ExternalInput")
    vb = nc.dram_tensor("vb", [P, D], F32, kind="ExternalInput")
    mb = nc.dram_tensor("mb", [LB, P, NKI], F32, kind="ExternalInput")
    out = nc.dram_tensor("out", [LB, S, D], BF, kind="ExternalOutput")

    with tile.TileContext(nc) as tc:
        _emit(nc, tc, x8, fvt, wq8, wk8, wv, qb, kb, vb, mb, out, vs)
    nc.finalize()
    return nc


def _emit(nc, tc, x8, fvt, wq8, wk8, wv, qb, kb, vb, mb, out, vs):
    from contextlib import ExitStack

    with ExitStack() as ctx:
        wpool = ctx.enter_context(tc.tile_pool(name="wpool", bufs=1))
        apool = ctx.enter_context(tc.tile_pool(name="apool", bufs=1))
        opool = ctx.enter_context(tc.tile_pool(name="opool", bufs=3))
        spool = ctx.enter_context(tc.tile_pool(name="spool", bufs=2))
        pp = ctx.enter_context(tc.tile_pool(name="pp", bufs=6, space="PSUM"))
        pd = ctx.enter_context(tc.tile_pool(name="pd", bufs=2, space="PSUM"))

        QB = wpool.tile([P, NDI], F32, tag="qb")
        nc.sync.dma_start(out=QB, in_=qb[:, :])
        KB = wpool.tile([P, NDI], F32, tag="kb")
        nc.sync.dma_start(out=KB, in_=kb[:, :])
        VB = wpool.tile([P, D], F32, tag="vb")
        nc.sync.dma_start(out=VB, in_=vb[:, :])
        ONES = wpool.tile([P, 1], F8, tag="ones")
        nc.vector.memset(ONES, 1.0)
        WQ8 = wpool.tile([P, NCI, D], F8, tag="wq8")
        WV = wpool.tile([P, NCI, D], BF, tag="wv")
        WK8 = None

        for b in range(LB):
            v = vs[b]  # valid key chunks for this batch slot
            # key-dim psum groups: equal-width pieces covering v*128 cols
            # (equal widths keep every group wide enough for DoubleRow)
            n_g = -(-v * P // NF)
            base = (v * P // n_g) // 32 * 32
            kg, off = [], 0
            for g in range(n_g):
                w = v * P - off if g == n_g - 1 else base
                kg.append((off, w))
                off += w

            # --- stage Q: QT8[d, s] (fp8 DoubleRow over ci pairs) --------
            X8 = apool.tile([P, NCI, SP8], F8, tag="x8")
            for ci in range(NCI):
                nc.sync.dma_start(out=X8[:, ci, :], in_=x8[b, ci * P:(ci + 1) * P, :])
                if b == 0:
                    # low di columns first so early psum groups can start
                    # before the whole 1 MiB of wq8 lands
                    nc.sync.dma_start(
                        out=WQ8[:, ci, :NF],
                        in_=wq8[ci * P:(ci + 1) * P, :NF])
            if b == 0:
                for ci in range(NCI):
                    nc.sync.dma_start(
                        out=WQ8[:, ci, NF:],
                        in_=wq8[ci * P:(ci + 1) * P, NF:])
            MB = spool.tile([P, NKI], F32, tag="mb")
            nc.sync.dma_start(out=MB, in_=mb[b])
            QT8 = apool.tile([P, NDI, S], F8, tag="qt8")
            for di in range(NDI):
                ps = [pp.tile([P, NF], F32, tag="ps", name=f"ps{_i}") for _i in range(2)]
                for c0 in range(0, NCI, 2):
                    lhsT = WQ8[:, c0:c0 + 2, di * P:(di + 1) * P]
                    for sh in range(2):
                        nc.tensor.matmul(
                            ps[sh], lhsT,
                            X8[:, c0:c0 + 2, PAD + sh * NF: PAD + sh * NF + NF],
                            start=(c0 == 0), stop=(c0 == NCI - 2), perf_mode=DR)
                for sh in range(2):
                    nc.scalar.activation(
                        QT8[:, di, sh * NF:(sh + 1) * NF], ps[sh], AF.Identity,
                        bias=QB[:, di:di + 1], scale=QSCALE)

            # --- stage K: KT8[d, s] (width-5 stencil, v key chunks) ------
            if WK8 is None:
                WK8 = []
                for j in range(5):
                    t = wpool.tile([P, NCI, D], F8, tag=f"wk8{j}")
                    # second HWDGE ring (Activation) in parallel with sync's
                    nc.scalar.dma_start(
                        out=t, in_=wk8[j].rearrange("(ci p) d -> p ci d", p=P))
                    WK8.append(t)
            KT8 = apool.tile([P, NDI, S], F8, tag="kt8")
            for di in range(NDI):
                ps = [pp.tile([P, NF], F32, tag="ps", name=f"ps{_i}")
                      for _i in range(len(kg))]
                # per-group matmul counters for start/stop bookkeeping
                ndr = [w >= 160 for (_, w) in kg]
                total = [(5 * NCI // 2) if d else 5 * NCI for d in ndr]
                done = [0] * len(kg)
                for j in range(5):
                    for c0 in range(0, NCI, 2):
                        for g, (off, w) in enumerate(kg):
                            if ndr[g]:
                                nc.tensor.matmul(
                                    ps[g][:, :w],
                                    WK8[j][:, c0:c0 + 2, di * P:(di + 1) * P],
                                    X8[:, c0:c0 + 2, j + off: j + off + w],
                                    start=(done[g] == 0),
                                    stop=(done[g] == total[g] - 1),
                                    perf_mode=DR)
                                done[g] += 1
                            else:
                                for cc in (c0, c0 + 1):
                                    nc.tensor.matmul(
                                        ps[g][:, :w],
                                        WK8[j][:, cc, di * P:(di + 1) * P],
                                        X8[:, cc, j + off: j + off + w],
                                        start=(done[g] == 0),
                                        stop=(done[g] == total[g] - 1))
                                    done[g] += 1
                for g, (off, w) in enumerate(kg):
                    nc.scalar.activation(
                        KT8[:, di, off:off + w], ps[g][:, :w], AF.Identity,
                        bias=KB[:, di:di + 1], scale=KSCALE)

            # --- stage E: ET8[k, q] = 2^4 exp(scoresT/32 + mask) ---------
            ET8 = apool.tile([P, NKI, S], F8, tag="et8")
            for ki in range(v):
                ps = [pp.tile([P, NF], F32, tag="ps", name=f"ps{_i}") for _i in range(2)]
                for d0 in range(0, NDI, 2):
                    lhsT = KT8[:, d0:d0 + 2, ki * P:(ki + 1) * P]
                    for qh in range(2):
                        nc.tensor.matmul(
                            ps[qh], lhsT, QT8[:, d0:d0 + 2, qh * NF:(qh + 1) * NF],
                            start=(d0 == 0), stop=(d0 == NDI - 2), perf_mode=DR)
                for qh in range(2):
                    nc.scalar.activation(
                        ET8[:, ki, qh * NF:(qh + 1) * NF], ps[qh], AF.Exp,
                        bias=MB[:, ki:ki + 1], scale=ESCALE)

            # --- stage V: V natural [s, d] (bf16) + fp8 copy for PV ------
            FVT = apool.tile([P, NCI, S], BF, tag="fvt")
            for ci in range(NCI):
                nc.sync.dma_start(out=FVT[:, ci, :], in_=fvt[b, ci * P:(ci + 1) * P, :])
                if b == 0:
                    nc.sync.dma_start(out=WV[:, ci, :], in_=wv[ci * P:(ci + 1) * P, :])
            V = apool.tile([P, NSI, D], BF, tag="v")
            V8 = apool.tile([P, NKI, D], F8, tag="v8")
            for si in range(NSI):
                ps = [pp.tile([P, NF], F32, tag="ps", name=f"ps{_i}") for _i in range(2)]
                for ci in range(NCI):
                    lhsT = FVT[:, ci, si * P:(si + 1) * P]
                    for dh in range(2):
                        nc.tensor.matmul(
                            ps[dh], lhsT, WV[:, ci, dh * NF:(dh + 1) * NF],
                            start=(ci == 0), stop=(ci == NCI - 1))
                for dh in range(2):
                    nc.vector.tensor_add(
                        V[:, si, dh * NF:(dh + 1) * NF], ps[dh],
                        VB[:, dh * NF:(dh + 1) * NF])
                if si < v:
                    for dh in range(2):
                        nc.scalar.activation(
                            V8[:, si, dh * NF:(dh + 1) * NF],
                            V[:, si, dh * NF:(dh + 1) * NF],
                            AF.Copy, bias=0.0, scale=1.0)

            # --- stage F: out = (ET^T @ V) / den + V ---------------------
            for qi in range(NQI):
                pso = [pp.tile([P, NF], F32, tag="ps", name=f"pso{_i}") for _i in range(2)]
                psd = pd.tile([P, 1], F32, tag="den")
                # den first: its tiny psum evacuates (reciprocal) while the
                # pso DoubleRow matmuls still run, so the pd bank recycles
                # without ever stalling the PE.
                for ki in range(v):
                    nc.tensor.matmul(
                        psd, ET8[:, ki, qi * P:(qi + 1) * P], ONES,
                        start=(ki == 0), stop=(ki == v - 1))
                REC = spool.tile([P, 1], F32, tag="rec")
                nc.vector.reciprocal(REC, psd)
                vev = v - (v % 2)
                for k0 in range(0, vev, 2):
                    lhsT = ET8[:, k0:k0 + 2, qi * P:(qi + 1) * P]
                    st, sp_ = (k0 == 0), (k0 + 2 >= v)
                    for dh in range(2):
                        nc.tensor.matmul(
                            pso[dh], lhsT, V8[:, k0:k0 + 2, dh * NF:(dh + 1) * NF],
                            start=st, stop=sp_, perf_mode=DR)
                if v % 2:
                    lhsT = ET8[:, v - 1, qi * P:(qi + 1) * P]
                    for dh in range(2):
                        nc.tensor.matmul(
                            pso[dh], lhsT, V8[:, v - 1, dh * NF:(dh + 1) * NF],
                            start=(v == 1), stop=True)
                # Free the PSUM banks with plain DVE copies that wait only on
                # the matmul stop; the reciprocal-scale and +V run in place on
                # SBUF afterwards, off the PE-critical path.
                OTs = []
                for dh in range(2):
                    OT = opool.tile([P, NF], F32, tag="out", name=f"ot{dh}")
                    nc.vector.tensor_copy(OT, pso[dh])
                    OTs.append(OT)
                for dh in range(2):
                    OT = OTs[dh]
                    nc.scalar.activation(
                        OT, OT, AF.Copy, bias=0.0, scale=REC)
                    OB = opool.tile([P, NF], BF, tag="outb", name=f"ob{dh}")
                    nc.vector.tensor_add(
                        OB, OT, V[:, qi, dh * NF:(dh + 1) * NF])
                    nc.scalar.dma_start(
                        out=out[b, qi * P:(qi + 1) * P, dh * NF:(dh + 1) * NF],
                        in_=OB)


def _prep_host(feaQK, feaV, seqlengths, cn3_w, cn3_b, cn5_w, cn5_b,
               k_w, k_b, q_w, q_b, v_w, v_b):
    """Compose weights, assign batches to cores, lay out per-core inputs."""
    f32 = np.float32
    bf16 = ml_dtypes.bfloat16
    feaQK = np.asarray(feaQK, f32)
    feaV = np.asarray(feaV, f32)
    seqlengths = np.asarray(seqlengths).astype(np.int64)

    W1 = np.asarray(k_w, f32)[:, :C]
    W2 = np.asarray(k_w, f32)[:, C:2 * C]
    W3 = np.asarray(k_w, f32)[:, 2 * C:]

    wk = np.zeros((5, C, D), f32)  # [tap j (= shift+2), c, d]
    for t in range(3):
        wk[t + 1] += (W2 @ np.asarray(cn3_w, f32)[:, :, t]).T
    for t in range(5):
        wk[t] += (W3 @ np.asarray(cn5_w, f32)[:, :, t]).T
    wk[2] += W1.T
    kb_eff = (np.asarray(k_b, f32) + W2 @ np.asarray(cn3_b, f32)
              + W3 @ np.asarray(cn5_b, f32))

    def q8(a, s):
        return np.clip(np.asarray(a, f32) * s, -F8MAX, F8MAX).astype(E4)

    wq8 = np.ascontiguousarray(q8(np.asarray(q_w, f32).T, SW))
    wk8 = np.ascontiguousarray(q8(wk, SW))
    wv_b = np.ascontiguousarray(np.asarray(v_w, f32).T).astype(bf16)

    qb_pd = np.ascontiguousarray(
        (np.asarray(q_b, f32) * SQ).reshape(NDI, P).T)
    kb_pd = np.ascontiguousarray((kb_eff * SK).reshape(NDI, P).T)
    vb_rep = np.ascontiguousarray(
        np.broadcast_to(np.asarray(v_b, f32), (P, D)))

    key_valid = np.arange(S)[None, :] < seqlengths[:, None]
    mask = np.where(key_valid, EEXP * np.log(2.0), MASK_NEG).astype(f32)

    # Pair longest with shortest so the compile-time per-slot chunk counts
    # (max over cores) stay near the per-core optimum.
    vchunks = np.clip(np.ceil(seqlengths / P).astype(int), 1, NKI)
    order = np.argsort(-seqlengths, kind="stable")
    batch_of = np.zeros((NCORES, LB), int)
    for i in range(NCORES):
        batch_of[i, 0] = order[B - 1 - i]
        batch_of[i, 1] = order[i]
    vs = (int(vchunks[batch_of[:, 0]].max()),
          int(vchunks[batch_of[:, 1]].max()))

    in_maps = []
    for core in range(NCORES):
        bs = batch_of[core]
        x8s = np.zeros((LB, C, SP8), E4)
        x8s[:, :, PAD:PAD + S] = q8(feaQK[bs].transpose(0, 2, 1), SX)
        fvts = np.ascontiguousarray(
            feaV[bs].transpose(0, 2, 1)).astype(bf16)
        mbs = np.ascontiguousarray(
            mask[bs].reshape(LB, NKI, P).transpose(0, 2, 1))
        in_maps.append({
            "x8": x8s, "fvt": fvts,
            "wq8": wq8, "wk8": wk8, "wv": wv_b,
            "qb": qb_pd, "kb": kb_pd, "vb": vb_rep, "mb": mbs,
        })
    return in_maps, batch_of, vs


def kernel(**inputs):
    from concourse.bass_utils import run_bass_kernel_spmd

    in_maps, batch_of, vs = _prep_host(**inputs)
    if _CACHE.get("vs") != vs:
        _CACHE["nc"] = _build_program(vs)
        _CACHE["vs"] = vs
    nc = _CACHE["nc"]
    res = run_bass_kernel_spmd(nc, in_maps, core_ids=list(range(NCORES)),
                               trace=TRACE)
    _CACHE["last_result"] = res
    full = np.zeros((B, S, D), np.float32)
    for core in range(NCORES):
        full[batch_of[core]] = res.results[core]["out"].astype(np.float32)
    return full


# revision 30
# speedup vs baseline: 1.0575x; 1.0380x over previous
"""Contextual attention kernel for Trainium2 (8 NeuronCores, data-parallel over batch).

Math (per batch b):
    Q = feaQK @ q_w.T + q_b
    k3 = conv1d(feaQK.T, cn3_w, SAME) + b3 ; k5 = conv1d(..., cn5_w) + b5
    K = [feaQK, k3, k5] @ k_w.T + k_b
    V = feaV @ v_w.T + v_b
    S = (Q @ K.T) / sqrt(D); mask keys >= seqlen with -inf
    out = softmax(S) @ V + V

Kernel strategy:
  * The convs + concat + K-projection collapse into a single width-5 stencil:
        K[s] = sum_{d=-2..2} feaQK[s+d] @ Wk[d] + kb_eff
    composed on the host (15 matmul-units of work -> 9).
  * All activations on-chip in transposed layout ([feature, seq]); no
    on-device transposes anywhere.
  * Q/K projections, scores, and PV run in fp8(e4m3) with
    perf_mode=DoubleRow: each matmul contracts 256 (2x128 chunk pairs) at
    ~2x bf16 ALU rate. Power-of-2 scale factors (exact in fp8) keep the
    tiny weights out of the subnormal range and every fp8-written tensor
    under the TRN e4m3 max of 240 (overflow would produce Inf, not
    saturation):
        x*2^4, wq/wk*2^12, QT/KT*2^5, ET*2^4 (folded into the exp bias as
        +4*ln2; cancels exactly against den = sum ET in the softmax ratio).
    Narrow psum groups (<256 cols) use plain fp8 matmuls (FWL beats
    DoubleRow's 256-col weight load there).
  * V projection stays bf16 (out ~= V + small attention average, so V's
    precision dominates the final error); V8 = fp8 copy of V feeds the PV
    moving operand.
  * Keys beyond seqlength are dead: K/scores/PV cover only the first
    ceil(seqlen/128) key chunks per batch slot; batches paired
    longest-with-shortest across cores keep the compile-time per-slot
    chunk counts small. Sub-chunk masking goes through the exp bias.
  * 16 batches -> 2 per core, full weights on every core.
"""

import numpy as np
import ml_dtypes

import concourse.bass as bass
from concourse import bacc
import concourse.tile as tile
from concourse import mybir

B, S, C, D = 16, 1024, 1024, 1024
P = 128
NCI, NDI, NKI, NQI, NSI = C // P, D // P, S // P, S // P, S // P
NF = 512  # matmul free dim (one PSUM bank of fp32)
PAD = 2
SP8 = 1040  # padded seq cols for fp8 x (mult of 16 for DR interleave APs)
LB = 2  # local batches per core
NCORES = 8
MASK_NEG = -60000.0
SCALE = 1.0 / 32.0  # 1/sqrt(D)

# fp8 power-of-2 scales (exact): see module docstring.
SX, SW, SQ, SK = 2.0**4, 2.0**12, 2.0**5, 2.0**5
EEXP = 4  # ET = 2^4 * exp(scores/32), via +EEXP*ln2 in the exp bias
QSCALE = SQ / (SX * SW)            # psum -> QT units
KSCALE = SK / (SX * SW)
ESCALE = SCALE / (SQ * SK)         # psum -> exp input
F8MAX = 240.0                      # TRN e4m3 max normal

BF = mybir.dt.bfloat16
F8 = mybir.dt.float8e4
F32 = mybir.dt.float32
AF = mybir.ActivationFunctionType
DR = mybir.MatmulPerfMode.DoubleRow
E4 = ml_dtypes.float8_e4m3

TRACE = False  # set by test harness to collect HW profile
_CACHE = {}


def _build_program(vs):
    nc = bacc.Bacc("TRN2", dynamic_dma_scratch_size=256)

    # All big inputs are host-permuted into the exact [partition, free]
    # SBUF layout, so each loads as ONE fully-contiguous DMA (few
    # descriptors, split across all 16 SDMA engines).
    x8 = nc.dram_tensor("x8", [LB, P, NCI * SP8], F8, kind="ExternalInput")
    fvt = nc.dram_tensor("fvt", [LB, P, NCI * S], BF, kind="ExternalInput")
    wq8 = nc.dram_tensor("wq8", [P, NCI * D], F8, kind="ExternalInput")
    wk8 = nc.dram_tensor("wk8", [5, P, NCI * D], F8, kind="ExternalInput")
    wv = nc.dram_tensor("wv", [P, NCI * D], BF, kind="ExternalInput")
    qb = nc.dram_tensor("qb", [P, NDI], F32, kind="ExternalInput")
    kb = nc.dram_tensor("kb", [P, NDI], F32, kind="ExternalInput")
    vb = nc.dram_tensor("vb", [P, D], F32, kind="ExternalInput")
    mb = nc.dram_tensor("mb", [LB, P, NKI], F32, kind="ExternalInput")
    out = nc.dram_tensor("out", [LB, S, D], BF, kind="ExternalOutput")

    with tile.TileContext(nc) as tc:
        _emit(nc, tc, x8, fvt, wq8, wk8, wv, qb, kb, vb, mb, out, vs)
    nc.finalize()
    return nc


def _emit(nc, tc, x8, fvt, wq8, wk8, wv, qb, kb, vb, mb, out, vs):
    from contextlib import ExitStack

    with ExitStack() as ctx:
        wpool = ctx.enter_context(tc.tile_pool(name="wpool", bufs=1))
        apool = ctx.enter_context(tc.tile_pool(name="apool", bufs=1))
        opool = ctx.enter_context(tc.tile_pool(name="opool", bufs=3))
        spool = ctx.enter_context(tc.tile_pool(name="spool", bufs=2))
        pp = ctx.enter_context(tc.tile_pool(name="pp", bufs=6, space="PSUM"))
        pd = ctx.enter_context(tc.tile_pool(name="pd", bufs=2, space="PSUM"))

        QB = wpool.tile([P, NDI], F32, tag="qb")
        nc.sync.dma_start(out=QB, in_=qb[:, :])
        KB = wpool.tile([P, NDI], F32, tag="kb")
        nc.sync.dma_start(out=KB, in_=kb[:, :])
        VB = wpool.tile([P, D], F32, tag="vb")
        nc.sync.dma_start(out=VB, in_=vb[:, :])
        ONES = wpool.tile([P, 1], F8, tag="ones")
        nc.vector.memset(ONES, 1.0)
        WQ8 = wpool.tile([P, NCI, D], F8, tag="wq8")
        WV = wpool.tile([P, NCI, D], BF, tag="wv")
        WK8 = None

        for b in range(LB):
            v = vs[b]  # valid key chunks for this batch slot
            # key-dim psum groups: equal-width pieces covering v*128 cols
            # (equal widths keep every group wide enough for DoubleRow)
            n_g = -(-v * P // NF)
            base = (v * P // n_g) // 32 * 32
            kg, off = [], 0
            for g in range(n_g):
                w = v * P - off if g == n_g - 1 else base
                kg.append((off, w))
                off += w

            # --- stage Q: QT8[d, s] (fp8 DoubleRow over ci pairs) --------
            # Alternate the two HWDGE rings (sync / scalar): each DMA runs
            # on a single ~24 GB/s SDMA engine, so parallelism comes from
            # many outstanding DMAs spread over both trigger queues.
            rings = [nc.sync, nc.scalar]
            X8 = apool.tile([P, NCI, SP8], F8, tag="x8")
            for ci in range(NCI):
                rings[ci % 2].dma_start(
                    out=X8[:, ci, :],
                    in_=x8[b, :, ci * SP8:(ci + 1) * SP8])
                if b == 0:
                    rings[(ci + 1) % 2].dma_start(
                        out=WQ8[:, ci, :],
                        in_=wq8[:, ci * D:(ci + 1) * D])
            MB = spool.tile([P, NKI], F32, tag="mb")
            nc.sync.dma_start(out=MB, in_=mb[b])
            QT8 = apool.tile([P, NDI, S], F8, tag="qt8")
            for di in range(NDI):
                ps = [pp.tile([P, NF], F32, tag="ps", name=f"ps{_i}") for _i in range(2)]
                for c0 in range(0, NCI, 2):
                    lhsT = WQ8[:, c0:c0 + 2, di * P:(di + 1) * P]
                    for sh in range(2):
                        nc.tensor.matmul(
                            ps[sh], lhsT,
                            X8[:, c0:c0 + 2, PAD + sh * NF: PAD + sh * NF + NF],
                            start=(c0 == 0), stop=(c0 == NCI - 2), perf_mode=DR)
                for sh in range(2):
                    nc.scalar.activation(
                        QT8[:, di, sh * NF:(sh + 1) * NF], ps[sh], AF.Identity,
                        bias=QB[:, di:di + 1], scale=QSCALE)

            # --- stage K: KT8[d, s] (width-5 stencil, v key chunks) ------
            if WK8 is None:
                WK8 = []
                for j in range(5):
                    t = wpool.tile([P, NCI, D], F8, tag=f"wk8{j}")
                    for cc in range(0, NCI, 4):
                        rings[(j + cc // 4) % 2].dma_start(
                            out=t[:, cc:cc + 4, :],
                            in_=wk8[j, :, cc * D:(cc + 4) * D])
                    WK8.append(t)
            KT8 = apool.tile([P, NDI, S], F8, tag="kt8")
            for di in range(NDI):
                ps = [pp.tile([P, NF], F32, tag="ps", name=f"ps{_i}")
                      for _i in range(len(kg))]
                # per-group matmul counters for start/stop bookkeeping
                ndr = [w >= 160 for (_, w) in kg]
                total = [(5 * NCI // 2) if d else 5 * NCI for d in ndr]
                done = [0] * len(kg)
                for j in range(5):
                    for c0 in range(0, NCI, 2):
                        for g, (off, w) in enumerate(kg):
                            if ndr[g]:
                                nc.tensor.matmul(
                                    ps[g][:, :w],
                                    WK8[j][:, c0:c0 + 2, di * P:(di + 1) * P],
                                    X8[:, c0:c0 + 2, j + off: j + off + w],
                                    start=(done[g] == 0),
                                    stop=(done[g] == total[g] - 1),
                                    perf_mode=DR)
                                done[g] += 1
                            else:
                                for cc in (c0, c0 + 1):
                                    nc.tensor.matmul(
                                        ps[g][:, :w],
                                        WK8[j][:, cc, di * P:(di + 1) * P],
                                        X8[:, cc, j + off: j + off + w],
                                        start=(done[g] == 0),
                                        stop=(done[g] == total[g] - 1))
                                    done[g] += 1
                for g, (off, w) in enumerate(kg):
                    nc.scalar.activation(
                        KT8[:, di, off:off + w], ps[g][:, :w], AF.Identity,
                        bias=KB[:, di:di + 1], scale=KSCALE)

            # --- stage V: V natural [s, d] (bf16) + fp8 copy for PV ------
            # (before stage E so E's scalar-evacuated psums sit between the
            # V matmuls and stage F -- avoids a psum-bank stall at F start)
            FVT = apool.tile([P, NCI, S], BF, tag="fvt")
            for ci in range(NCI):
                rings[ci % 2].dma_start(
                    out=FVT[:, ci, :], in_=fvt[b, :, ci * S:(ci + 1) * S])
                if b == 0:
                    rings[(ci + 1) % 2].dma_start(
                        out=WV[:, ci, :], in_=wv[:, ci * D:(ci + 1) * D])
            V = apool.tile([P, NSI, D], BF, tag="v")
            V8 = apool.tile([P, NKI, D], F8, tag="v8")
            for si in range(NSI):
                ps = [pp.tile([P, NF], F32, tag="ps", name=f"ps{_i}") for _i in range(2)]
                for ci in range(NCI):
                    lhsT = FVT[:, ci, si * P:(si + 1) * P]
                    for dh in range(2):
                        nc.tensor.matmul(
                            ps[dh], lhsT, WV[:, ci, dh * NF:(dh + 1) * NF],
                            start=(ci == 0), stop=(ci == NCI - 1))
                for dh in range(2):
                    nc.vector.tensor_add(
                        V[:, si, dh * NF:(dh + 1) * NF], ps[dh],
                        VB[:, dh * NF:(dh + 1) * NF])
                if si < v:
                    for dh in range(2):
                        nc.scalar.activation(
                            V8[:, si, dh * NF:(dh + 1) * NF],
                            V[:, si, dh * NF:(dh + 1) * NF],
                            AF.Copy, bias=0.0, scale=1.0)

            # --- stage E: ET8[k, q] = 2^4 exp(scoresT/32 + mask) ---------
            ET8 = apool.tile([P, NKI, S], F8, tag="et8")
            for ki in range(v):
                ps = [pp.tile([P, NF], F32, tag="ps", name=f"ps{_i}") for _i in range(2)]
                for d0 in range(0, NDI, 2):
                    lhsT = KT8[:, d0:d0 + 2, ki * P:(ki + 1) * P]
                    for qh in range(2):
                        nc.tensor.matmul(
                            ps[qh], lhsT, QT8[:, d0:d0 + 2, qh * NF:(qh + 1) * NF],
                            start=(d0 == 0), stop=(d0 == NDI - 2), perf_mode=DR)
                for qh in range(2):
                    nc.scalar.activation(
                        ET8[:, ki, qh * NF:(qh + 1) * NF], ps[qh], AF.Exp,
                        bias=MB[:, ki:ki + 1], scale=ESCALE)

            # --- stage F: out = (ET^T @ V) / den + V ---------------------
            for qi in range(NQI):
                pso = [pp.tile([P, NF], F32, tag="ps", name=f"pso{_i}") for _i in range(2)]
                psd = pd.tile([P, 1], F32, tag="den")
                # den first: its tiny psum evacuates (reciprocal) while the
                # pso DoubleRow matmuls still run, so the pd bank recycles
                # without ever stalling the PE.
                for ki in range(v):
                    nc.tensor.matmul(
                        psd, ET8[:, ki, qi * P:(qi + 1) * P], ONES,
                        start=(ki == 0), stop=(ki == v - 1))
                REC = spool.tile([P, 1], F32, tag="rec")
                nc.vector.reciprocal(REC, psd)
                vev = v - (v % 2)
                for k0 in range(0, vev, 2):
                    lhsT = ET8[:, k0:k0 + 2, qi * P:(qi + 1) * P]
                    st, sp_ = (k0 == 0), (k0 + 2 >= v)
                    for dh in range(2):
                        nc.tensor.matmul(
                            pso[dh], lhsT, V8[:, k0:k0 + 2, dh * NF:(dh + 1) * NF],
                            start=st, stop=sp_, perf_mode=DR)
                if v % 2:
                    lhsT = ET8[:, v - 1, qi * P:(qi + 1) * P]
                    for dh in range(2):
                        nc.tensor.matmul(
                            pso[dh], lhsT, V8[:, v - 1, dh * NF:(dh + 1) * NF],
                            start=(v == 1), stop=True)
                # Free the PSUM banks with plain DVE copies that wait only on
                # the matmul stop; the reciprocal-scale and +V run in place on
                # SBUF afterwards, off the PE-critical path.
                OTs = []
                for dh in range(2):
                    OT = opool.tile([P, NF], F32, tag="out", name=f"ot{dh}")
                    nc.vector.tensor_copy(OT, pso[dh])
                    OTs.append(OT)
                for dh in range(2):
                    OT = OTs[dh]
                    nc.scalar.activation(
                        OT, OT, AF.Copy, bias=0.0, scale=REC)
                    OB = opool.tile([P, NF], BF, tag="outb", name=f"ob{dh}")
                    nc.vector.tensor_add(
                        OB, OT, V[:, qi, dh * NF:(dh + 1) * NF])
                    rings[dh].dma_start(
                        out=out[b, qi * P:(qi + 1) * P, dh * NF:(dh + 1) * NF],
                        in_=OB)


def _prep_host(feaQK, feaV, seqlengths, cn3_w, cn3_b, cn5_w, cn5_b,
               k_w, k_b, q_w, q_b, v_w, v_b):
    """Compose weights, assign batches to cores, lay out per-core inputs."""
    f32 = np.float32
    bf16 = ml_dtypes.bfloat16
    feaQK = np.asarray(feaQK, f32)
    feaV = np.asarray(feaV, f32)
    seqlengths = np.asarray(seqlengths).astype(np.int64)

    W1 = np.asarray(k_w, f32)[:, :C]
    W2 = np.asarray(k_w, f32)[:, C:2 * C]
    W3 = np.asarray(k_w, f32)[:, 2 * C:]

    wk = np.zeros((5, C, D), f32)  # [tap j (= shift+2), c, d]
    for t in range(3):
        wk[t + 1] += (W2 @ np.asarray(cn3_w, f32)[:, :, t]).T
    for t in range(5):
        wk[t] += (W3 @ np.asarray(cn5_w, f32)[:, :, t]).T
    wk[2] += W1.T
    kb_eff = (np.asarray(k_b, f32) + W2 @ np.asarray(cn3_b, f32)
              + W3 @ np.asarray(cn5_b, f32))

    def q8(a, s):
        return np.clip(np.asarray(a, f32) * s, -F8MAX, F8MAX).astype(E4)

    def perm(a):
        # [C, X] -> [P, NCI*X]: partition-major layout matching the SBUF
        # tile, so the load is one fully-contiguous DMA
        X = a.shape[-1]
        return np.ascontiguousarray(
            a.reshape(NCI, P, X).transpose(1, 0, 2).reshape(P, NCI * X))

    wq8 = perm(q8(np.asarray(q_w, f32).T, SW))
    wk8 = np.stack([perm(q8(wk[j], SW)) for j in range(5)])
    wv_b = perm(np.asarray(v_w, f32).T.astype(bf16))

    qb_pd = np.ascontiguousarray(
        (np.asarray(q_b, f32) * SQ).reshape(NDI, P).T)
    kb_pd = np.ascontiguousarray((kb_eff * SK).reshape(NDI, P).T)
    vb_rep = np.ascontiguousarray(
        np.broadcast_to(np.asarray(v_b, f32), (P, D)))

    key_valid = np.arange(S)[None, :] < seqlengths[:, None]
    mask = np.where(key_valid, EEXP * np.log(2.0), MASK_NEG).astype(f32)

    # Pair longest with shortest so the compile-time per-slot chunk counts
    # (max over cores) stay near the per-core optimum.
    vchunks = np.clip(np.ceil(seqlengths / P).astype(int), 1, NKI)
    order = np.argsort(-seqlengths, kind="stable")
    batch_of = np.zeros((NCORES, LB), int)
    for i in range(NCORES):
        batch_of[i, 0] = order[B - 1 - i]
        batch_of[i, 1] = order[i]
    vs = (int(vchunks[batch_of[:, 0]].max()),
          int(vchunks[batch_of[:, 1]].max()))

    in_maps = []
    for core in range(NCORES):
        bs = batch_of[core]
        x8s = np.zeros((LB, C, SP8), E4)
        x8s[:, :, PAD:PAD + S] = q8(feaQK[bs].transpose(0, 2, 1), SX)
        x8s = np.stack([perm(x8s[i]) for i in range(LB)])
        fvts = np.stack([
            perm(np.ascontiguousarray(feaV[i].T).astype(bf16)) for i in bs])
        mbs = np.ascontiguousarray(
            mask[bs].reshape(LB, NKI, P).transpose(0, 2, 1))
        in_maps.append({
            "x8": x8s, "fvt": fvts,
            "wq8": wq8, "wk8": wk8, "wv": wv_b,
            "qb": qb_pd, "kb": kb_pd, "vb": vb_rep, "mb": mbs,
        })
    return in_maps, batch_of, vs


def kernel(**inputs):
    from concourse.bass_utils import run_bass_kernel_spmd

    in_maps, batch_of, vs = _prep_host(**inputs)
    if _CACHE.get("vs") != vs:
        _CACHE["nc"] = _build_program(vs)
        _CACHE["vs"] = vs
    nc = _CACHE["nc"]
    res = run_bass_kernel_spmd(nc, in_maps, core_ids=list(range(NCORES)),
                               trace=TRACE)
    _CACHE["last_result"] = res
    full = np.zeros((B, S, D), np.float32)
    for core in range(NCORES):
        full[batch_of[core]] = res.results[core]["out"].astype(np.float32)
    return full


# revision 32
# speedup vs baseline: 1.0660x; 1.0080x over previous
"""Contextual attention kernel for Trainium2 (8 NeuronCores, data-parallel over batch).

Math (per batch b):
    Q = feaQK @ q_w.T + q_b
    k3 = conv1d(feaQK.T, cn3_w, SAME) + b3 ; k5 = conv1d(..., cn5_w) + b5
    K = [feaQK, k3, k5] @ k_w.T + k_b
    V = feaV @ v_w.T + v_b
    S = (Q @ K.T) / sqrt(D); mask keys >= seqlen with -inf
    out = softmax(S) @ V + V

Kernel strategy:
  * The convs + concat + K-projection collapse into a single width-5 stencil:
        K[s] = sum_{d=-2..2} feaQK[s+d] @ Wk[d] + kb_eff
    composed on the host (15 matmul-units of work -> 9).
  * All activations on-chip in transposed layout ([feature, seq]); no
    on-device transposes anywhere.
  * Q/K projections, scores, and PV run in fp8(e4m3) with
    perf_mode=DoubleRow: each matmul contracts 256 (2x128 chunk pairs) at
    ~2x bf16 ALU rate. Power-of-2 scale factors (exact in fp8) keep the
    tiny weights out of the subnormal range and every fp8-written tensor
    under the TRN e4m3 max of 240 (overflow would produce Inf, not
    saturation):
        x*2^4, wq/wk*2^12, QT/KT*2^5, ET*2^4 (folded into the exp bias as
        +4*ln2; cancels exactly against den = sum ET in the softmax ratio).
    Narrow psum groups (<256 cols) use plain fp8 matmuls (FWL beats
    DoubleRow's 256-col weight load there).
  * V projection stays bf16 (out ~= V + small attention average, so V's
    precision dominates the final error); V8 = fp8 copy of V feeds the PV
    moving operand.
  * Keys beyond seqlength are dead: K/scores/PV cover only the first
    ceil(seqlen/128) key chunks per batch slot; batches paired
    longest-with-shortest across cores keep the compile-time per-slot
    chunk counts small. Sub-chunk masking goes through the exp bias.
  * 16 batches -> 2 per core, full weights on every core.
"""

import numpy as np
import ml_dtypes

import concourse.bass as bass
from concourse import bacc
import concourse.tile as tile
from concourse import mybir

B, S, C, D = 16, 1024, 1024, 1024
P = 128
NCI, NDI, NKI, NQI, NSI = C // P, D // P, S // P, S // P, S // P
NF = 512  # matmul free dim (one PSUM bank of fp32)
PAD = 2
SP8 = 1040  # padded seq cols for fp8 x (mult of 16 for DR interleave APs)
LB = 2  # local batches per core
NCORES = 8
MASK_NEG = -60000.0
SCALE = 1.0 / 32.0  # 1/sqrt(D)

# fp8 power-of-2 scales (exact): see module docstring.
SX, SW, SQ, SK = 2.0**4, 2.0**12, 2.0**5, 2.0**5
EEXP = 4  # ET = 2^4 * exp(scores/32), via +EEXP*ln2 in the exp bias
QSCALE = SQ / (SX * SW)            # psum -> QT units
KSCALE = SK / (SX * SW)
ESCALE = SCALE / (SQ * SK)         # psum -> exp input
F8MAX = 240.0                      # TRN e4m3 max normal

BF = mybir.dt.bfloat16
F8 = mybir.dt.float8e4
F32 = mybir.dt.float32
AF = mybir.ActivationFunctionType
DR = mybir.MatmulPerfMode.DoubleRow
E4 = ml_dtypes.float8_e4m3

TRACE = False  # set by test harness to collect HW profile
_CACHE = {}


def _build_program(vs):
    nc = bacc.Bacc("TRN2", dynamic_dma_scratch_size=256)

    # All big inputs are host-permuted into the exact [partition, free]
    # SBUF layout, so each loads as ONE fully-contiguous DMA (few
    # descriptors, split across all 16 SDMA engines).
    x8 = nc.dram_tensor("x8", [LB, P, NCI * SP8], F8, kind="ExternalInput")
    fvt = nc.dram_tensor("fvt", [LB, P, NCI * S], BF, kind="ExternalInput")
    wq8 = nc.dram_tensor("wq8", [P, NCI * D], F8, kind="ExternalInput")
    wk8 = nc.dram_tensor("wk8", [5, P, NCI * D], F8, kind="ExternalInput")
    wv = nc.dram_tensor("wv", [P, NCI * D], BF, kind="ExternalInput")
    qb = nc.dram_tensor("qb", [P, NDI], F32, kind="ExternalInput")
    kb = nc.dram_tensor("kb", [P, NDI], F32, kind="ExternalInput")
    vb = nc.dram_tensor("vb", [P, D], F32, kind="ExternalInput")
    mb = nc.dram_tensor("mb", [LB, P, NKI], F32, kind="ExternalInput")
    out = nc.dram_tensor("out", [LB, S, D], BF, kind="ExternalOutput")

    with tile.TileContext(nc) as tc:
        _emit(nc, tc, x8, fvt, wq8, wk8, wv, qb, kb, vb, mb, out, vs)
    nc.finalize()
    return nc


def _emit(nc, tc, x8, fvt, wq8, wk8, wv, qb, kb, vb, mb, out, vs):
    from contextlib import ExitStack

    with ExitStack() as ctx:
        wpool = ctx.enter_context(tc.tile_pool(name="wpool", bufs=1))
        apool = ctx.enter_context(tc.tile_pool(name="apool", bufs=1))
        opool = ctx.enter_context(tc.tile_pool(name="opool", bufs=3))
        spool = ctx.enter_context(tc.tile_pool(name="spool", bufs=2))
        pp = ctx.enter_context(tc.tile_pool(name="pp", bufs=6, space="PSUM"))
        pd = ctx.enter_context(tc.tile_pool(name="pd", bufs=2, space="PSUM"))

        QB = wpool.tile([P, NDI], F32, tag="qb")
        nc.sync.dma_start(out=QB, in_=qb[:, :])
        KB = wpool.tile([P, NDI], F32, tag="kb")
        nc.sync.dma_start(out=KB, in_=kb[:, :])
        VB = wpool.tile([P, D], F32, tag="vb")
        nc.sync.dma_start(out=VB, in_=vb[:, :])
        ONES = wpool.tile([P, 1], F8, tag="ones")
        nc.vector.memset(ONES, 1.0)
        WQ8 = wpool.tile([P, NCI, D], F8, tag="wq8")
        WV = wpool.tile([P, NCI, D], BF, tag="wv")
        WK8 = None

        for b in range(LB):
            v = vs[b]  # valid key chunks for this batch slot
            # key-dim psum groups: equal-width pieces covering v*128 cols
            # (equal widths keep every group wide enough for DoubleRow)
            n_g = -(-v * P // NF)
            base = (v * P // n_g) // 32 * 32
            kg, off = [], 0
            for g in range(n_g):
                w = v * P - off if g == n_g - 1 else base
                kg.append((off, w))
                off += w

            # --- stage Q: QT8[d, s] (fp8 DoubleRow over ci pairs) --------
            # Alternate the two HWDGE rings (sync / scalar): each DMA runs
            # on a single ~24 GB/s SDMA engine, so parallelism comes from
            # many outstanding DMAs spread over both trigger queues.
            rings = [nc.sync, nc.scalar]
            X8 = apool.tile([P, NCI, SP8], F8, tag="x8")
            for ci in range(NCI):
                rings[ci % 2].dma_start(
                    out=X8[:, ci, :],
                    in_=x8[b, :, ci * SP8:(ci + 1) * SP8])
                if b == 0:
                    rings[(ci + 1) % 2].dma_start(
                        out=WQ8[:, ci, :],
                        in_=wq8[:, ci * D:(ci + 1) * D])
            MB = spool.tile([P, NKI], F32, tag="mb")
            nc.sync.dma_start(out=MB, in_=mb[b])
            QT8 = apool.tile([P, NDI, S], F8, tag="qt8")
            for di in range(NDI):
                ps = [pp.tile([P, NF], F32, tag="ps", name=f"ps{_i}") for _i in range(2)]
                for c0 in range(0, NCI, 2):
                    lhsT = WQ8[:, c0:c0 + 2, di * P:(di + 1) * P]
                    for sh in range(2):
                        nc.tensor.matmul(
                            ps[sh], lhsT,
                            X8[:, c0:c0 + 2, PAD + sh * NF: PAD + sh * NF + NF],
                            start=(c0 == 0), stop=(c0 == NCI - 2), perf_mode=DR)
                for sh in range(2):
                    nc.scalar.activation(
                        QT8[:, di, sh * NF:(sh + 1) * NF], ps[sh], AF.Identity,
                        bias=QB[:, di:di + 1], scale=QSCALE)

            # --- stage K: KT8[d, s] (width-5 stencil, v key chunks) ------
            if WK8 is None:
                WK8 = []
                for j in range(5):
                    t = wpool.tile([P, NCI, D], F8, tag=f"wk8{j}")
                    # ~256 KiB pieces keep both rings' 8 outstanding-DMA
                    # lanes full (each DMA moves at only ~24 GB/s)
                    for cc in range(0, NCI, 2):
                        rings[(j + cc // 2) % 2].dma_start(
                            out=t[:, cc:cc + 2, :],
                            in_=wk8[j, :, cc * D:(cc + 2) * D])
                    WK8.append(t)
            KT8 = apool.tile([P, NDI, S], F8, tag="kt8")
            for di in range(NDI):
                ps = [pp.tile([P, NF], F32, tag="ps", name=f"ps{_i}")
                      for _i in range(len(kg))]
                # per-group matmul counters for start/stop bookkeeping
                ndr = [w >= 160 for (_, w) in kg]
                total = [(5 * NCI // 2) if d else 5 * NCI for d in ndr]
                done = [0] * len(kg)
                for j in range(5):
                    for c0 in range(0, NCI, 2):
                        for g, (off, w) in enumerate(kg):
                            if ndr[g]:
                                nc.tensor.matmul(
                                    ps[g][:, :w],
                                    WK8[j][:, c0:c0 + 2, di * P:(di + 1) * P],
                                    X8[:, c0:c0 + 2, j + off: j + off + w],
                                    start=(done[g] == 0),
                                    stop=(done[g] == total[g] - 1),
                                    perf_mode=DR)
                                done[g] += 1
                            else:
                                for cc in (c0, c0 + 1):
                                    nc.tensor.matmul(
                                        ps[g][:, :w],
                                        WK8[j][:, cc, di * P:(di + 1) * P],
                                        X8[:, cc, j + off: j + off + w],
                                        start=(done[g] == 0),
                                        stop=(done[g] == total[g] - 1))
                                    done[g] += 1
                for g, (off, w) in enumerate(kg):
                    nc.scalar.activation(
                        KT8[:, di, off:off + w], ps[g][:, :w], AF.Identity,
                        bias=KB[:, di:di + 1], scale=KSCALE)

            # --- stage V: V natural [s, d] (bf16) + fp8 copy for PV ------
            # (before stage E so E's scalar-evacuated psums sit between the
            # V matmuls and stage F -- avoids a psum-bank stall at F start)
            FVT = apool.tile([P, NCI, S], BF, tag="fvt")
            for ci in range(NCI):
                rings[ci % 2].dma_start(
                    out=FVT[:, ci, :], in_=fvt[b, :, ci * S:(ci + 1) * S])
                if b == 0:
                    rings[(ci + 1) % 2].dma_start(
                        out=WV[:, ci, :], in_=wv[:, ci * D:(ci + 1) * D])
            V = apool.tile([P, NSI, D], BF, tag="v")
            V8 = apool.tile([P, NKI, D], F8, tag="v8")
            for si in range(NSI):
                ps = [pp.tile([P, NF], F32, tag="ps", name=f"ps{_i}") for _i in range(2)]
                for ci in range(NCI):
                    lhsT = FVT[:, ci, si * P:(si + 1) * P]
                    for dh in range(2):
                        nc.tensor.matmul(
                            ps[dh], lhsT, WV[:, ci, dh * NF:(dh + 1) * NF],
                            start=(ci == 0), stop=(ci == NCI - 1))
                for dh in range(2):
                    nc.vector.tensor_add(
                        V[:, si, dh * NF:(dh + 1) * NF], ps[dh],
                        VB[:, dh * NF:(dh + 1) * NF])
                if si < v:
                    for dh in range(2):
                        nc.scalar.activation(
                            V8[:, si, dh * NF:(dh + 1) * NF],
                            V[:, si, dh * NF:(dh + 1) * NF],
                            AF.Copy, bias=0.0, scale=1.0)

            # --- stage E: ET8[k, q] = 2^4 exp(scoresT/32 + mask) ---------
            ET8 = apool.tile([P, NKI, S], F8, tag="et8")
            for ki in range(v):
                ps = [pp.tile([P, NF], F32, tag="ps", name=f"ps{_i}") for _i in range(2)]
                for d0 in range(0, NDI, 2):
                    lhsT = KT8[:, d0:d0 + 2, ki * P:(ki + 1) * P]
                    for qh in range(2):
                        nc.tensor.matmul(
                            ps[qh], lhsT, QT8[:, d0:d0 + 2, qh * NF:(qh + 1) * NF],
                            start=(d0 == 0), stop=(d0 == NDI - 2), perf_mode=DR)
                for qh in range(2):
                    nc.scalar.activation(
                        ET8[:, ki, qh * NF:(qh + 1) * NF], ps[qh], AF.Exp,
                        bias=MB[:, ki:ki + 1], scale=ESCALE)

            # --- stage F: out = (ET^T @ V) / den + V ---------------------
            for qi in range(NQI):
                pso = [pp.tile([P, NF], F32, tag="ps", name=f"pso{_i}") for _i in range(2)]
                psd = pd.tile([P, 1], F32, tag="den")
                # den first: its tiny psum evacuates (reciprocal) while the
                # pso DoubleRow matmuls still run, so the pd bank recycles
                # without ever stalling the PE.
                for ki in range(v):
                    nc.tensor.matmul(
                        psd, ET8[:, ki, qi * P:(qi + 1) * P], ONES,
                        start=(ki == 0), stop=(ki == v - 1))
                REC = spool.tile([P, 1], F32, tag="rec")
                nc.vector.reciprocal(REC, psd)
                vev = v - (v % 2)
                for k0 in range(0, vev, 2):
                    lhsT = ET8[:, k0:k0 + 2, qi * P:(qi + 1) * P]
                    st, sp_ = (k0 == 0), (k0 + 2 >= v)
                    for dh in range(2):
                        nc.tensor.matmul(
                            pso[dh], lhsT, V8[:, k0:k0 + 2, dh * NF:(dh + 1) * NF],
                            start=st, stop=sp_, perf_mode=DR)
                if v % 2:
                    lhsT = ET8[:, v - 1, qi * P:(qi + 1) * P]
                    for dh in range(2):
                        nc.tensor.matmul(
                            pso[dh], lhsT, V8[:, v - 1, dh * NF:(dh + 1) * NF],
                            start=(v == 1), stop=True)
                # REC is ready before pso stops (den ran first), so the
                # scale-activation can evacuate the psum directly; the +V
                # add and the quarter-split output DMAs pipeline behind it.
                for dh in range(2):
                    OT = opool.tile([P, NF], F32, tag="out", name=f"ot{dh}")
                    nc.scalar.activation(
                        OT, pso[dh], AF.Copy, bias=0.0, scale=REC)
                    OB = opool.tile([P, NF], BF, tag="outb", name=f"ob{dh}")
                    nc.vector.tensor_add(
                        OB, OT, V[:, qi, dh * NF:(dh + 1) * NF])
                    for hh in range(2):
                        ring = nc.scalar if (dh, hh) == (1, 1) else nc.sync
                        cl, ch = hh * (NF // 2), (hh + 1) * (NF // 2)
                        ring.dma_start(
                            out=out[b, qi * P:(qi + 1) * P,
                                    dh * NF + cl: dh * NF + ch],
                            in_=OB[:, cl:ch])


def _prep_host(feaQK, feaV, seqlengths, cn3_w, cn3_b, cn5_w, cn5_b,
               k_w, k_b, q_w, q_b, v_w, v_b):
    """Compose weights, assign batches to cores, lay out per-core inputs."""
    f32 = np.float32
    bf16 = ml_dtypes.bfloat16
    feaQK = np.asarray(feaQK, f32)
    feaV = np.asarray(feaV, f32)
    seqlengths = np.asarray(seqlengths).astype(np.int64)

    W1 = np.asarray(k_w, f32)[:, :C]
    W2 = np.asarray(k_w, f32)[:, C:2 * C]
    W3 = np.asarray(k_w, f32)[:, 2 * C:]

    wk = np.zeros((5, C, D), f32)  # [tap j (= shift+2), c, d]
    for t in range(3):
        wk[t + 1] += (W2 @ np.asarray(cn3_w, f32)[:, :, t]).T
    for t in range(5):
        wk[t] += (W3 @ np.asarray(cn5_w, f32)[:, :, t]).T
    wk[2] += W1.T
    kb_eff = (np.asarray(k_b, f32) + W2 @ np.asarray(cn3_b, f32)
              + W3 @ np.asarray(cn5_b, f32))

    def q8(a, s):
        return np.clip(np.asarray(a, f32) * s, -F8MAX, F8MAX).astype(E4)

    def perm(a):
        # [C, X] -> [P, NCI*X]: partition-major layout matching the SBUF
        # tile, so the load is one fully-contiguous DMA
        X = a.shape[-1]
        return np.ascontiguousarray(
            a.reshape(NCI, P, X).transpose(1, 0, 2).reshape(P, NCI * X))

    wq8 = perm(q8(np.asarray(q_w, f32).T, SW))
    wk8 = np.stack([perm(q8(wk[j], SW)) for j in range(5)])
    wv_b = perm(np.asarray(v_w, f32).T.astype(bf16))

    qb_pd = np.ascontiguousarray(
        (np.asarray(q_b, f32) * SQ).reshape(NDI, P).T)
    kb_pd = np.ascontiguousarray((kb_eff * SK).reshape(NDI, P).T)
    vb_rep = np.ascontiguousarray(
        np.broadcast_to(np.asarray(v_b, f32), (P, D)))

    key_valid = np.arange(S)[None, :] < seqlengths[:, None]
    mask = np.where(key_valid, EEXP * np.log(2.0), MASK_NEG).astype(f32)

    # Pair longest with shortest so the compile-time per-slot chunk counts
    # (max over cores) stay near the per-core optimum.
    vchunks = np.clip(np.ceil(seqlengths / P).astype(int), 1, NKI)
    order = np.argsort(-seqlengths, kind="stable")
    batch_of = np.zeros((NCORES, LB), int)
    for i in range(NCORES):
        batch_of[i, 0] = order[B - 1 - i]
        batch_of[i, 1] = order[i]
    vs = (int(vchunks[batch_of[:, 0]].max()),
          int(vchunks[batch_of[:, 1]].max()))

    in_maps = []
    for core in range(NCORES):
        bs = batch_of[core]
        x8s = np.zeros((LB, C, SP8), E4)
        x8s[:, :, PAD:PAD + S] = q8(feaQK[bs].transpose(0, 2, 1), SX)
        x8s = np.stack([perm(x8s[i]) for i in range(LB)])
        fvts = np.stack([
            perm(np.ascontiguousarray(feaV[i].T).astype(bf16)) for i in bs])
        mbs = np.ascontiguousarray(
            mask[bs].reshape(LB, NKI, P).transpose(0, 2, 1))
        in_maps.append({
            "x8": x8s, "fvt": fvts,
            "wq8": wq8, "wk8": wk8, "wv": wv_b,
            "qb": qb_pd, "kb": kb_pd, "vb": vb_rep, "mb": mbs,
        })
    return in_maps, batch_of, vs


def kernel(**inputs):
    from concourse.bass_utils import run_bass_kernel_spmd

    in_maps, batch_of, vs = _prep_host(**inputs)
    if _CACHE.get("vs") != vs:
        _CACHE["nc"] = _build_program(vs)
        _CACHE["vs"] = vs
    nc = _CACHE["nc"]
    res = run_bass_kernel_spmd(nc, in_maps, core_ids=list(range(NCORES)),
                               trace=TRACE)
    _CACHE["last_result"] = res
    full = np.zeros((B, S, D), np.float32)
    for core in range(NCORES):
        full[batch_of[core]] = res.results[core]["out"].astype(np.float32)
    return full


# revision 36
# speedup vs baseline: 1.0694x; 1.0032x over previous
"""Contextual attention kernel for Trainium2 (8 NeuronCores, data-parallel over batch).

Math (per batch b):
    Q = feaQK @ q_w.T + q_b
    k3 = conv1d(feaQK.T, cn3_w, SAME) + b3 ; k5 = conv1d(..., cn5_w) + b5
    K = [feaQK, k3, k5] @ k_w.T + k_b
    V = feaV @ v_w.T + v_b
    S = (Q @ K.T) / sqrt(D); mask keys >= seqlen with -inf
    out = softmax(S) @ V + V

Kernel strategy:
  * The convs + concat + K-projection collapse into a single width-5 stencil:
        K[s] = sum_{d=-2..2} feaQK[s+d] @ Wk[d] + kb_eff
    composed on the host (15 matmul-units of work -> 9).
  * All activations on-chip in transposed layout ([feature, seq]); no
    on-device transposes anywhere.
  * Q/K projections, scores, and PV run in fp8(e4m3) with
    perf_mode=DoubleRow: each matmul contracts 256 (2x128 chunk pairs) at
    ~2x bf16 ALU rate. Power-of-2 scale factors (exact in fp8) keep the
    tiny weights out of the subnormal range and every fp8-written tensor
    under the TRN e4m3 max of 240 (overflow would produce Inf, not
    saturation):
        x*2^4, wq/wk*2^12, QT/KT*2^5, ET*2^4 (folded into the exp bias as
        +4*ln2; cancels exactly against den = sum ET in the softmax ratio).
    Narrow psum groups (<256 cols) use plain fp8 matmuls (FWL beats
    DoubleRow's 256-col weight load there).
  * V projection stays bf16 (out ~= V + small attention average, so V's
    precision dominates the final error); V8 = fp8 copy of V feeds the PV
    moving operand.
  * Keys beyond seqlength are dead: K/scores/PV cover only the first
    ceil(seqlen/128) key chunks per batch slot; batches paired
    longest-with-shortest across cores keep the compile-time per-slot
    chunk counts small. Sub-chunk masking goes through the exp bias.
  * 16 batches -> 2 per core, full weights on every core.
"""

import numpy as np
import ml_dtypes

import concourse.bass as bass
from concourse import bacc
import concourse.tile as tile
from concourse import mybir

B, S, C, D = 16, 1024, 1024, 1024
P = 128
NCI, NDI, NKI, NQI, NSI = C // P, D // P, S // P, S // P, S // P
NF = 512  # matmul free dim (one PSUM bank of fp32)
PAD = 2
SP8 = 1040  # padded seq cols for fp8 x (mult of 16 for DR interleave APs)
LB = 2  # local batches per core
NCORES = 8
MASK_NEG = -60000.0
SCALE = 1.0 / 32.0  # 1/sqrt(D)

# fp8 power-of-2 scales (exact): see module docstring.
SX, SW, SQ, SK = 2.0**4, 2.0**12, 2.0**5, 2.0**5
EEXP = 4  # ET = 2^4 * exp(scores/32), via +EEXP*ln2 in the exp bias
QSCALE = SQ / (SX * SW)            # psum -> QT units
KSCALE = SK / (SX * SW)
ESCALE = SCALE / (SQ * SK)         # psum -> exp input
F8MAX = 240.0                      # TRN e4m3 max normal

BF = mybir.dt.bfloat16
F8 = mybir.dt.float8e4
F32 = mybir.dt.float32
AF = mybir.ActivationFunctionType
DR = mybir.MatmulPerfMode.DoubleRow
E4 = ml_dtypes.float8_e4m3

TRACE = False  # set by test harness to collect HW profile
_CACHE = {}


def _build_program(vs):
    nc = bacc.Bacc("TRN2", dynamic_dma_scratch_size=256)

    # All big inputs are host-permuted into the exact [partition, free]
    # SBUF layout, so each loads as ONE fully-contiguous DMA (few
    # descriptors, split across all 16 SDMA engines).
    x8 = nc.dram_tensor("x8", [LB, P, NCI * SP8], F8, kind="ExternalInput")
    fvt = nc.dram_tensor("fvt", [LB, P, NCI * S], BF, kind="ExternalInput")
    wq8 = nc.dram_tensor("wq8", [P, NCI * D], F8, kind="ExternalInput")
    wk8 = nc.dram_tensor("wk8", [5, P, NCI * D], F8, kind="ExternalInput")
    wv = nc.dram_tensor("wv", [P, NCI * D], BF, kind="ExternalInput")
    qb = nc.dram_tensor("qb", [P, NDI], F32, kind="ExternalInput")
    kb = nc.dram_tensor("kb", [P, NDI], F32, kind="ExternalInput")
    vb = nc.dram_tensor("vb", [P, D], F32, kind="ExternalInput")
    mb = nc.dram_tensor("mb", [LB, P, NKI], F32, kind="ExternalInput")
    out = nc.dram_tensor("out", [LB, S, D], BF, kind="ExternalOutput")

    with tile.TileContext(nc) as tc:
        _emit(nc, tc, x8, fvt, wq8, wk8, wv, qb, kb, vb, mb, out, vs)
    nc.finalize()
    return nc


def _emit(nc, tc, x8, fvt, wq8, wk8, wv, qb, kb, vb, mb, out, vs):
    from contextlib import ExitStack

    with ExitStack() as ctx:
        wpool = ctx.enter_context(tc.tile_pool(name="wpool", bufs=1))
        apool = ctx.enter_context(tc.tile_pool(name="apool", bufs=1))
        dpool = ctx.enter_context(tc.tile_pool(name="dpool", bufs=2))
        opool = ctx.enter_context(tc.tile_pool(name="opool", bufs=3))
        spool = ctx.enter_context(tc.tile_pool(name="spool", bufs=2))
        pp = ctx.enter_context(tc.tile_pool(name="pp", bufs=6, space="PSUM"))
        pd = ctx.enter_context(tc.tile_pool(name="pd", bufs=2, space="PSUM"))

        rings = [nc.sync, nc.scalar]

        def load_x8(bb):
            t = dpool.tile([P, NCI, SP8], F8, tag="x8", name=f"x8b{bb}")
            for ci in range(NCI):
                rings[ci % 2].dma_start(
                    out=t[:, ci, :], in_=x8[bb, :, ci * SP8:(ci + 1) * SP8])
            return t

        def load_fvt(bb):
            t = dpool.tile([P, NCI, S], BF, tag="fvt", name=f"fvtb{bb}")
            for ci in range(NCI):
                rings[ci % 2].dma_start(
                    out=t[:, ci, :], in_=fvt[bb, :, ci * S:(ci + 1) * S])
            return t

        X8s, FVTs = {}, {}

        QB = wpool.tile([P, NDI], F32, tag="qb")
        nc.sync.dma_start(out=QB, in_=qb[:, :])
        KB = wpool.tile([P, NDI], F32, tag="kb")
        nc.sync.dma_start(out=KB, in_=kb[:, :])
        VB = wpool.tile([P, D], F32, tag="vb")
        nc.sync.dma_start(out=VB, in_=vb[:, :])
        ONES = wpool.tile([P, 1], F8, tag="ones")
        nc.vector.memset(ONES, 1.0)
        WQ8 = wpool.tile([P, NCI, D], F8, tag="wq8")
        WV = wpool.tile([P, NCI, D], BF, tag="wv")
        WK8 = None

        for b in range(LB):
            v = vs[b]  # valid key chunks for this batch slot
            # key-dim psum groups: equal-width pieces covering v*128 cols
            # (equal widths keep every group wide enough for DoubleRow)
            n_g = -(-v * P // NF)
            base = (v * P // n_g) // 32 * 32
            kg, off = [], 0
            for g in range(n_g):
                w = v * P - off if g == n_g - 1 else base
                kg.append((off, w))
                off += w

            # --- stage Q: QT8[d, s] (fp8 DoubleRow over ci pairs) --------
            # Alternate the two HWDGE rings (sync / scalar): each DMA runs
            # on a single ~24 GB/s SDMA engine, so parallelism comes from
            # many outstanding DMAs spread over both trigger queues.
            if b not in X8s:
                X8s[b] = load_x8(b)
                if b == 0:
                    for ci in range(NCI):
                        rings[(ci + 1) % 2].dma_start(
                            out=WQ8[:, ci, :],
                            in_=wq8[:, ci * D:(ci + 1) * D])
            X8 = X8s[b]
            MB = spool.tile([P, NKI], F32, tag="mb")
            nc.sync.dma_start(out=MB, in_=mb[b])
            QT8 = apool.tile([P, NDI, S], F8, tag="qt8")
            for di in range(NDI):
                ps = [pp.tile([P, NF], F32, tag="ps", name=f"ps{_i}") for _i in range(2)]
                for c0 in range(0, NCI, 2):
                    lhsT = WQ8[:, c0:c0 + 2, di * P:(di + 1) * P]
                    for sh in range(2):
                        nc.tensor.matmul(
                            ps[sh], lhsT,
                            X8[:, c0:c0 + 2, PAD + sh * NF: PAD + sh * NF + NF],
                            start=(c0 == 0), stop=(c0 == NCI - 2), perf_mode=DR)
                for sh in range(2):
                    nc.scalar.activation(
                        QT8[:, di, sh * NF:(sh + 1) * NF], ps[sh], AF.Identity,
                        bias=QB[:, di:di + 1], scale=QSCALE)

            # --- stage K: KT8[d, s] (width-5 stencil, v key chunks) ------
            if WK8 is None:
                WK8 = []
                for j in range(5):
                    t = wpool.tile([P, NCI, D], F8, tag=f"wk8{j}")
                    # ~256 KiB pieces keep both rings' 8 outstanding-DMA
                    # lanes full (each DMA moves at only ~24 GB/s)
                    for cc in range(0, NCI, 2):
                        rings[(j + cc // 2) % 2].dma_start(
                            out=t[:, cc:cc + 2, :],
                            in_=wk8[j, :, cc * D:(cc + 2) * D])
                    WK8.append(t)
            KT8 = apool.tile([P, NDI, S], F8, tag="kt8")
            for di in range(NDI):
                ps = [pp.tile([P, NF], F32, tag="ps", name=f"ps{_i}")
                      for _i in range(len(kg))]
                # per-group matmul counters for start/stop bookkeeping
                ndr = [w >= 160 for (_, w) in kg]
                total = [(5 * NCI // 2) if d else 5 * NCI for d in ndr]
                done = [0] * len(kg)
                for j in range(5):
                    for c0 in range(0, NCI, 2):
                        for g, (off, w) in enumerate(kg):
                            if ndr[g]:
                                nc.tensor.matmul(
                                    ps[g][:, :w],
                                    WK8[j][:, c0:c0 + 2, di * P:(di + 1) * P],
                                    X8[:, c0:c0 + 2, j + off: j + off + w],
                                    start=(done[g] == 0),
                                    stop=(done[g] == total[g] - 1),
                                    perf_mode=DR)
                                done[g] += 1
                            else:
                                for cc in (c0, c0 + 1):
                                    nc.tensor.matmul(
                                        ps[g][:, :w],
                                        WK8[j][:, cc, di * P:(di + 1) * P],
                                        X8[:, cc, j + off: j + off + w],
                                        start=(done[g] == 0),
                                        stop=(done[g] == total[g] - 1))
                                    done[g] += 1
                for g, (off, w) in enumerate(kg):
                    nc.scalar.activation(
                        KT8[:, di, off:off + w], ps[g][:, :w], AF.Identity,
                        bias=KB[:, di:di + 1], scale=KSCALE)

            # --- stage V: V natural [s, d] (bf16) + fp8 copy for PV ------
            # (before stage E so E's scalar-evacuated psums sit between the
            # V matmuls and stage F -- avoids a psum-bank stall at F start)
            if b not in FVTs:
                FVTs[b] = load_fvt(b)
                if b == 0:
                    for ci in range(NCI):
                        rings[(ci + 1) % 2].dma_start(
                            out=WV[:, ci, :], in_=wv[:, ci * D:(ci + 1) * D])
            FVT = FVTs[b]
            V = apool.tile([P, NSI, D], BF, tag="v")
            V8 = apool.tile([P, NKI, D], F8, tag="v8")
            for si in range(NSI):
                ps = [pp.tile([P, NF], F32, tag="ps", name=f"ps{_i}") for _i in range(2)]
                for ci in range(NCI):
                    lhsT = FVT[:, ci, si * P:(si + 1) * P]
                    for dh in range(2):
                        nc.tensor.matmul(
                            ps[dh], lhsT, WV[:, ci, dh * NF:(dh + 1) * NF],
                            start=(ci == 0), stop=(ci == NCI - 1))
                for dh in range(2):
                    nc.vector.tensor_add(
                        V[:, si, dh * NF:(dh + 1) * NF], ps[dh],
                        VB[:, dh * NF:(dh + 1) * NF])
                if si < v:
                    for dh in range(2):
                        nc.scalar.activation(
                            V8[:, si, dh * NF:(dh + 1) * NF],
                            V[:, si, dh * NF:(dh + 1) * NF],
                            AF.Copy, bias=0.0, scale=1.0)

            # --- stage E: ET8[k, q] = 2^4 exp(scoresT/32 + mask) ---------
            # Prefetch next batch's activations here: stage E has no DMA
            # traffic of its own, so the 3 MiB lands before stage F's
            # output DMAs need the rings (double-buffered tiles, no WAR).
            if b + 1 < LB:
                X8s[b + 1] = load_x8(b + 1)
                FVTs[b + 1] = load_fvt(b + 1)
            ET8 = apool.tile([P, NKI, S], F8, tag="et8")
            for ki in range(v):
                ps = [pp.tile([P, NF], F32, tag="ps", name=f"ps{_i}") for _i in range(2)]
                for d0 in range(0, NDI, 2):
                    lhsT = KT8[:, d0:d0 + 2, ki * P:(ki + 1) * P]
                    for qh in range(2):
                        nc.tensor.matmul(
                            ps[qh], lhsT, QT8[:, d0:d0 + 2, qh * NF:(qh + 1) * NF],
                            start=(d0 == 0), stop=(d0 == NDI - 2), perf_mode=DR)
                for qh in range(2):
                    nc.scalar.activation(
                        ET8[:, ki, qh * NF:(qh + 1) * NF], ps[qh], AF.Exp,
                        bias=MB[:, ki:ki + 1], scale=ESCALE)

            # --- stage F: out = (ET^T @ V) / den + V ---------------------
            for qi in range(NQI):
                pso = [pp.tile([P, NF], F32, tag="ps", name=f"pso{_i}") for _i in range(2)]
                psd = pd.tile([P, 1], F32, tag="den")
                # den first: its tiny psum evacuates (reciprocal) while the
                # pso DoubleRow matmuls still run, so the pd bank recycles
                # without ever stalling the PE.
                for ki in range(v):
                    nc.tensor.matmul(
                        psd, ET8[:, ki, qi * P:(qi + 1) * P], ONES,
                        start=(ki == 0), stop=(ki == v - 1))
                REC = spool.tile([P, 1], F32, tag="rec")
                nc.vector.reciprocal(REC, psd)
                vev = v - (v % 2)
                for k0 in range(0, vev, 2):
                    lhsT = ET8[:, k0:k0 + 2, qi * P:(qi + 1) * P]
                    st, sp_ = (k0 == 0), (k0 + 2 >= v)
                    for dh in range(2):
                        nc.tensor.matmul(
                            pso[dh], lhsT, V8[:, k0:k0 + 2, dh * NF:(dh + 1) * NF],
                            start=st, stop=sp_, perf_mode=DR)
                if v % 2:
                    lhsT = ET8[:, v - 1, qi * P:(qi + 1) * P]
                    for dh in range(2):
                        nc.tensor.matmul(
                            pso[dh], lhsT, V8[:, v - 1, dh * NF:(dh + 1) * NF],
                            start=(v == 1), stop=True)
                # REC is ready before pso stops (den ran first), so the
                # scale-activation can evacuate the psum directly; the +V
                # add and the quarter-split output DMAs pipeline behind it.
                for dh in range(2):
                    OT = opool.tile([P, NF], F32, tag="out", name=f"ot{dh}")
                    nc.scalar.activation(
                        OT, pso[dh], AF.Copy, bias=0.0, scale=REC)
                    OB = opool.tile([P, NF], BF, tag="outb", name=f"ob{dh}")
                    nc.vector.tensor_add(
                        OB, OT, V[:, qi, dh * NF:(dh + 1) * NF])
                    for hh in range(2):
                        ring = nc.scalar if (dh, hh) == (1, 1) else nc.sync
                        cl, ch = hh * (NF // 2), (hh + 1) * (NF // 2)
                        ring.dma_start(
                            out=out[b, qi * P:(qi + 1) * P,
                                    dh * NF + cl: dh * NF + ch],
                            in_=OB[:, cl:ch])


def _prep_host(feaQK, feaV, seqlengths, cn3_w, cn3_b, cn5_w, cn5_b,
               k_w, k_b, q_w, q_b, v_w, v_b):
    """Compose weights, assign batches to cores, lay out per-core inputs."""
    f32 = np.float32
    bf16 = ml_dtypes.bfloat16
    feaQK = np.asarray(feaQK, f32)
    feaV = np.asarray(feaV, f32)
    seqlengths = np.asarray(seqlengths).astype(np.int64)

    W1 = np.asarray(k_w, f32)[:, :C]
    W2 = np.asarray(k_w, f32)[:, C:2 * C]
    W3 = np.asarray(k_w, f32)[:, 2 * C:]

    wk = np.zeros((5, C, D), f32)  # [tap j (= shift+2), c, d]
    for t in range(3):
        wk[t + 1] += (W2 @ np.asarray(cn3_w, f32)[:, :, t]).T
    for t in range(5):
        wk[t] += (W3 @ np.asarray(cn5_w, f32)[:, :, t]).T
    wk[2] += W1.T
    kb_eff = (np.asarray(k_b, f32) + W2 @ np.asarray(cn3_b, f32)
              + W3 @ np.asarray(cn5_b, f32))

    def q8(a, s):
        return np.clip(np.asarray(a, f32) * s, -F8MAX, F8MAX).astype(E4)

    def perm(a):
        # [C, X] -> [P, NCI*X]: partition-major layout matching the SBUF
        # tile, so the load is one fully-contiguous DMA
        X = a.shape[-1]
        return np.ascontiguousarray(
            a.reshape(NCI, P, X).transpose(1, 0, 2).reshape(P, NCI * X))

    wq8 = perm(q8(np.asarray(q_w, f32).T, SW))
    wk8 = np.stack([perm(q8(wk[j], SW)) for j in range(5)])
    wv_b = perm(np.asarray(v_w, f32).T.astype(bf16))

    qb_pd = np.ascontiguousarray(
        (np.asarray(q_b, f32) * SQ).reshape(NDI, P).T)
    kb_pd = np.ascontiguousarray((kb_eff * SK).reshape(NDI, P).T)
    vb_rep = np.ascontiguousarray(
        np.broadcast_to(np.asarray(v_b, f32), (P, D)))

    key_valid = np.arange(S)[None, :] < seqlengths[:, None]
    mask = np.where(key_valid, EEXP * np.log(2.0), MASK_NEG).astype(f32)

    # Pair longest with shortest so the compile-time per-slot chunk counts
    # (max over cores) stay near the per-core optimum.
    vchunks = np.clip(np.ceil(seqlengths / P).astype(int), 1, NKI)
    order = np.argsort(-seqlengths, kind="stable")
    batch_of = np.zeros((NCORES, LB), int)
    for i in range(NCORES):
        batch_of[i, 0] = order[B - 1 - i]
        batch_of[i, 1] = order[i]
    vs = (int(vchunks[batch_of[:, 0]].max()),
          int(vchunks[batch_of[:, 1]].max()))

    in_maps = []
    for core in range(NCORES):
        bs = batch_of[core]
        x8s = np.zeros((LB, C, SP8), E4)
        x8s[:, :, PAD:PAD + S] = q8(feaQK[bs].transpose(0, 2, 1), SX)
        x8s = np.stack([perm(x8s[i]) for i in range(LB)])
        fvts = np.stack([
            perm(np.ascontiguousarray(feaV[i].T).astype(bf16)) for i in bs])
        mbs = np.ascontiguousarray(
            mask[bs].reshape(LB, NKI, P).transpose(0, 2, 1))
        in_maps.append({
            "x8": x8s, "fvt": fvts,
            "wq8": wq8, "wk8": wk8, "wv": wv_b,
            "qb": qb_pd, "kb": kb_pd, "vb": vb_rep, "mb": mbs,
        })
    return in_maps, batch_of, vs


def kernel(**inputs):
    from concourse.bass_utils import run_bass_kernel_spmd

    in_maps, batch_of, vs = _prep_host(**inputs)
    if _CACHE.get("vs") != vs:
        _CACHE["nc"] = _build_program(vs)
        _CACHE["vs"] = vs
    nc = _CACHE["nc"]
    res = run_bass_kernel_spmd(nc, in_maps, core_ids=list(range(NCORES)),
                               trace=TRACE)
    _CACHE["last_result"] = res
    full = np.zeros((B, S, D), np.float32)
    for core in range(NCORES):
        full[batch_of[core]] = res.results[core]["out"].astype(np.float32)
    return full
